# revision 6
# baseline (speedup 1.0000x reference)
"""Grouped-index Conv1D (moe_routing) on 8 TRN2 NeuronCores.

Math:  out[b,d,t] = sum_c sum_k x[b,c,t+k] * W[gi[b,c],d,k] + count0[b]*bias[d]

Device algorithm (per core, 2 batches, data-parallel over batch), bf16:
  1. host precomputes one-hot M[c, 16*(2b+h)+g] = (gi[b,128h+c]==g) in bf16,
     stacked conv weights ws[16k+g, d] = W[g,d,k] (k=7 row zero) in bf16, and
     bc[d, b] = count0[b]*bias[d] in f32.  x is host-cast to bf16 (tolerance
     2e-2 >> bf16 rounding), halving HBM read traffic.
  2. S[g,t] = sum_c M[c,g]*x[c,t] (PE one-hot matmul, contraction 2x128) for
     both batches lands in ONE joint tile s4[0:16, 2048*b + t].
  3. quad shift replication via 4 SBUF->SBUF DMAs total (k=7 weight is 0):
       s4[16j+g, t] = S[g, t+j]        j=1..3   (3 DMAs, read rows 0-15)
       sw4[16j+g, t] = s4[16j+g, t+4]  j=0..2   (1 DMA, k=4..6)
  4. out[d,t] = wsA^T @ s4[:, t:t+L] + wsB^T @ sw4[:, t:t+L]
     (2 PSUM-accumulated matmuls, contraction 64 + 48, N<=512)
  5. bias add fused into PSUM->SBUF evacuation (ACT/DVE alternating),
     output stored bf16, one DMA per batch on the gpsimd (SWDGE) queue.

DMA-issue instructions cost a flat ~600 ns each on the issuing engine's
queue, so the design minimizes dma_start count (v1: 43 -> 25 us serialized
issue; v3: 10 issues split across the Sync and GpSimd queues).  SBUF APs
may only cross partitions in dim 0, which is why the shift replication
needs the two-level quad decomposition instead of one fused DMA.
"""

import sys
import numpy as np

sys.path.insert(0, "/opt/trn_rl_repo")

import ml_dtypes

BS, CH, T = 16, 256, 2048
G, D, K = 16, 64, 7
T_OUT = T - K + 1  # 2042
N_CORES = 8
BPC = BS // N_CORES  # batches per core = 2

BF16 = ml_dtypes.bfloat16

CONV_CHUNKS = [(0, 512), (512, 1024), (1024, 1536), (1536, 2042)]

MM_DTYPE = "bf16"

_COMPILED = {}


def _build(cfg: str):
    from concourse import bacc, tile
    import concourse.mybir as mybir

    f32 = mybir.dt.float32
    bf16 = mybir.dt.bfloat16
    add = mybir.AluOpType.add
    act_id = mybir.ActivationFunctionType.Identity
    act_copy = mybir.ActivationFunctionType.Copy

    nc = bacc.Bacc("TRN2", target_bir_lowering=False, debug=False,
                   num_devices=N_CORES)
    # x layout: [b, q, p, h*1024+t']  where channel = 128*h + p and
    # global col = 1024*q + t'  (one 512 KB DMA per (b, q))
    x_ext = nc.dram_tensor("x", [BPC, 2, 128, 2048], bf16,
                           kind="ExternalInput").ap()
    # cs: cols 0-63 one-hot M (col 16*(2b+h)+g), cols 64-127 wsA (k=0..3,
    # rows 0-63), cols 128-191 wsB (k=4..6, rows 0-47)
    cs_ext = nc.dram_tensor("cs", [128, 192], bf16, kind="ExternalInput").ap()
    bc_ext = nc.dram_tensor("bc", [D, BPC], f32, kind="ExternalInput").ap()
    out_ext = nc.dram_tensor("out", [BPC, D, T_OUT], bf16,
                             kind="ExternalOutput").ap()

    with tile.TileContext(nc) as tc:
        with (
            tc.tile_pool(name="const", bufs=1) as cpool,
            tc.tile_pool(name="work", bufs=2) as wpool,
            tc.tile_pool(name="ps_pool", bufs=2, space="PSUM") as ppool,
            tc.tile_pool(name="po_pool", bufs=4, space="PSUM") as opool,
        ):
            cs_sb = cpool.tile([128, 192], bf16, name="cs_sb")
            nc.sync.dma_start(cs_sb[:], cs_ext[:])
            bc_sb = cpool.tile([D, BPC], f32, name="bc_sb")
            nc.sync.dma_start(bc_sb[:], bc_ext[:])

            # --- x loads: 4 transfers of 512 KB ---
            xts = [[None, None] for _ in range(BPC)]
            for b in range(BPC):
                for q in range(2):
                    t_ = wpool.tile([128, 2048], bf16, name=f"xt{b}{q}",
                                    tag="xt", bufs=4)
                    nc.sync.dma_start(t_[:], x_ext[b, q])
                    xts[b][q] = t_

            # --- S stage into the joint tile s4[0:16, 2048b + t] ---
            s4 = wpool.tile([4 * G, 2 * T], bf16, name="s4", tag="s4")
            for b in range(BPC):
                for q in range(2):
                    ps = ppool.tile([G, 1024], f32, name=f"ps{b}{q}", tag="ps")
                    for cc in range(2):
                        sl = slice(512 * cc, 512 * cc + 512)
                        for h in range(2):
                            nc.tensor.matmul(
                                ps[:, sl],
                                cs_sb[:, G * (2 * b + h):G * (2 * b + h + 1)],
                                xts[b][q][:, 1024 * h + 512 * cc:
                                          1024 * h + 512 * cc + 512],
                                start=(h == 0), stop=(h == 1))
                    # evacuate PSUM -> SBUF (cast bf16), alternating engines
                    dst = s4[0:G, 2048 * b + 1024 * q:2048 * b + 1024 * (q + 1)]
                    if q == 0:
                        nc.vector.tensor_copy(dst, ps[:])
                    else:
                        nc.scalar.activation(dst, ps[:], act_copy)

            # --- shift replication: 4 SBUF->SBUF DMAs for BOTH batches ---
            for j in range(1, 4):
                nc.sync.dma_start(s4[G * j:G * (j + 1), 0:2 * T - j],
                                  s4[0:G, j:2 * T])
            sw4 = wpool.tile([3 * G, 2 * T], bf16, name="sw4", tag="sw4")
            nc.sync.dma_start(sw4[:, 0:2 * T - 6], s4[0:3 * G, 4:2 * T - 2])

            # --- conv + bias + store ---
            for b in range(BPC):
                osb = wpool.tile([D, T_OUT], bf16, name=f"osb{b}", tag="osb")
                for ci, (c0, c1) in enumerate(CONV_CHUNKS):
                    L = c1 - c0
                    po = opool.tile([D, 512], f32, name=f"po{b}{ci}", tag="po")
                    nc.tensor.matmul(po[:, :L], cs_sb[0:4 * G, 64:128],
                                     s4[:, 2048 * b + c0:2048 * b + c1],
                                     start=True, stop=False)
                    nc.tensor.matmul(po[:, :L], cs_sb[0:3 * G, 128:192],
                                     sw4[:, 2048 * b + c0:2048 * b + c1],
                                     start=False, stop=True)
                    if ci % 2 == 0:
                        nc.scalar.activation(osb[:, c0:c1], po[:, :L], act_id,
                                             bias=bc_sb[:, b:b + 1])
                    else:
                        nc.vector.tensor_scalar(out=osb[:, c0:c1],
                                                in0=po[:, :L],
                                                scalar1=bc_sb[:, b:b + 1],
                                                scalar2=None, op0=add)
                nc.gpsimd.dma_start(out_ext[b], osb[:])

    nc.compile()
    return nc


def _get_nc(mm_dtype: str):
    if mm_dtype not in _COMPILED:
        _COMPILED[mm_dtype] = _build(mm_dtype)
    return _COMPILED[mm_dtype]


def _run(x, group_idxs, W, bias, mm_dtype=None, trace=False, tmpdir=None):
    from concourse.bass_utils import run_bass_kernel_spmd

    x = np.asarray(x, dtype=np.float32)
    gi = np.asarray(group_idxs)
    W = np.asarray(W, dtype=np.float32)
    bias = np.asarray(bias, dtype=np.float32)

    # x per core: [2, 256, 2048] -> [b, h, p, q, t'] -> [b, q, p, h, t']
    xr = x.reshape(BS // BPC, BPC, 2, 128, 2, 1024).transpose(0, 1, 4, 3, 2, 5)
    xr = np.ascontiguousarray(xr.reshape(BS // BPC, BPC, 2, 128, 2048)
                              ).astype(BF16)
    # one-hot M: [bs, ch] -> per core [128, 4*G] with col (2b+h)*G+g
    oh = (gi[..., None] == np.arange(G)).astype(np.float32)  # [bs, 256, 16]
    ohm = oh.reshape(BS // BPC, BPC, 2, 128, G).transpose(0, 3, 1, 2, 4)
    ohm = ohm.reshape(BS // BPC, 128, 4 * G)
    # wsA[16j+g, d] = W[g, d, j] (j=0..3); wsB[16j+g, d] = W[g, d, 4+j]
    wsk = W.transpose(2, 0, 1).reshape(K * G, D)
    wsA = np.zeros((128, D), dtype=np.float32)
    wsA[:4 * G] = wsk[:4 * G]
    wsB = np.zeros((128, D), dtype=np.float32)
    wsB[:3 * G] = wsk[4 * G:]
    cs = np.concatenate([ohm, np.broadcast_to(wsA, (BS // BPC, 128, D)),
                         np.broadcast_to(wsB, (BS // BPC, 128, D))],
                        axis=2).astype(BF16)  # [cores, 128, 192]
    # bc[d, b] = count0[b] * bias[d]
    count0 = (gi == 0).sum(axis=1).astype(np.float32)  # [bs]
    bc = (count0[None, :] * bias[:, None]).astype(np.float32)  # [64, bs]
    bc = bc.reshape(D, BS // BPC, BPC).transpose(1, 0, 2)  # [cores, 64, 2]

    nc = _get_nc(mm_dtype or MM_DTYPE)
    in_maps = []
    for i in range(N_CORES):
        in_maps.append({
            "x": xr[i],
            "cs": np.ascontiguousarray(cs[i]),
            "bc": np.ascontiguousarray(bc[i]),
        })
    res = run_bass_kernel_spmd(nc, in_maps, core_ids=list(range(N_CORES)),
                               trace=trace, tmpdir=tmpdir)
    out = np.concatenate([np.asarray(r["out"], dtype=np.float32)
                          for r in res.results], axis=0)
    assert out.shape == (BS, D, T_OUT)
    return out, res


def kernel(x, group_idxs, W, bias):
    out, _ = _run(x, group_idxs, W, bias)
    return out


# revision 7
# speedup vs baseline: 1.4131x; 1.4131x over previous
"""Grouped-index Conv1D (moe_routing) on 8 TRN2 NeuronCores.

Math:  out[b,d,t] = sum_c sum_k x[b,c,t+k] * W[gi[b,c],d,k] + count0[b]*bias[d]

Device algorithm (per core, 2 batches, data-parallel over batch), bf16:
  1. host precomputes one-hot M[c, 16*(2b+h)+g] = (gi[b,128h+c]==g) in bf16,
     stacked conv weights ws[16k+g, d] = W[g,d,k] in bf16, and
     bc[d, b] = count0[b]*bias[d] in f32.  x is host-cast to bf16 (tolerance
     2e-2 >> bf16 rounding), halving HBM read traffic.
  2. ~8 warmup matmuls on a memset scratch tile run during the fixed ~7 us
     framework preamble + x-load time so the PE HAM clock gate is at 2.4 GHz
     (not the cold 1.2) when real matmuls start.
  3. S[g,t] = sum_c M[c,g]*x[c,t] (PE one-hot matmul, contraction 2x128),
     per-batch tile s[16, 2048], PSUM evacuated in [16,512] chunks on
     alternating DVE/ACT.
  4. swin[16k+g, t] = S[g, t+k]: 7 shifted SBUF->SBUF DMAs per batch (k=0..6
     full width), split 4 on the Sync HWDGE queue + 3 on the GpSimd SWDGE
     queue (DMA-issue instructions cost a flat ~600 ns on their queue; the
     two queues run in parallel and are otherwise idle at that point).
  5. out[d,t] = ws^T @ swin[:, t:t+L] (single matmul, contraction 112,
     N<=512); bias add fused into the PSUM->SBUF evacuation (ACT/DVE
     alternating); output stored bf16, one DMA per batch on GpSimd.

Stage-major emission (S b0, S b1, shifts, conv b0, conv b1) keeps the PE
queue dense: batch 1's S matmuls fill the batch-0 shift-DMA latency.
"""

import sys
import numpy as np

sys.path.insert(0, "/opt/trn_rl_repo")

import ml_dtypes

BS, CH, T = 16, 256, 2048
G, D, K = 16, 64, 7
T_OUT = T - K + 1  # 2042
N_CORES = 8
BPC = BS // N_CORES  # batches per core = 2

BF16 = ml_dtypes.bfloat16

CONV_CHUNKS = [(0, 512), (512, 1024), (1024, 1536), (1536, 2042)]
N_WARMUP = 8

MM_DTYPE = "bf16"

_COMPILED = {}


def _build(cfg: str):
    from concourse import bacc, tile
    import concourse.mybir as mybir

    f32 = mybir.dt.float32
    bf16 = mybir.dt.bfloat16
    add = mybir.AluOpType.add
    act_id = mybir.ActivationFunctionType.Identity
    act_copy = mybir.ActivationFunctionType.Copy

    nc = bacc.Bacc("TRN2", target_bir_lowering=False, debug=False,
                   num_devices=N_CORES)
    # x layout: [b, q, p, h*1024+t']  where channel = 128*h + p and
    # global col = 1024*q + t'  (one 512 KB DMA per (b, q))
    x_ext = nc.dram_tensor("x", [BPC, 2, 128, 2048], bf16,
                           kind="ExternalInput").ap()
    # cs: cols 0-63 one-hot M (col 16*(2b+h)+g), cols 64-127 ws (rows 0-111)
    cs_ext = nc.dram_tensor("cs", [128, 128], bf16, kind="ExternalInput").ap()
    bc_ext = nc.dram_tensor("bc", [D, BPC], f32, kind="ExternalInput").ap()
    out_ext = nc.dram_tensor("out", [BPC, D, T_OUT], bf16,
                             kind="ExternalOutput").ap()

    with tile.TileContext(nc) as tc:
        with (
            tc.tile_pool(name="const", bufs=1) as cpool,
            tc.tile_pool(name="work", bufs=2) as wpool,
            tc.tile_pool(name="ps_pool", bufs=4, space="PSUM") as ppool,
            tc.tile_pool(name="po_pool", bufs=4, space="PSUM") as opool,
        ):
            # --- PE warmup: memset scratch, then dummy matmuls to trip the
            # HAM clock gate to 8/8 before the real matmuls arrive ---
            scr = cpool.tile([128, 512], bf16, name="scr")
            nc.gpsimd.memset(scr[:], 0.0)
            for w in range(N_WARMUP):
                pw = opool.tile([G, 512], f32, name=f"pw{w}", tag="po")
                nc.tensor.matmul(pw[:], scr[:, 0:G], scr[:],
                                 start=True, stop=True)

            cs_sb = cpool.tile([128, 128], bf16, name="cs_sb")
            nc.scalar.dma_start(cs_sb[:], cs_ext[:])
            bc_sb = cpool.tile([D, BPC], f32, name="bc_sb")
            nc.scalar.dma_start(bc_sb[:], bc_ext[:])

            # --- x loads: 4 transfers of 512 KB on the Sync queue ---
            xts = [[None, None] for _ in range(BPC)]
            for b in range(BPC):
                for q in range(2):
                    t_ = wpool.tile([128, 2048], bf16, name=f"xt{b}{q}",
                                    tag="xt", bufs=4)
                    nc.sync.dma_start(t_[:], x_ext[b, q])
                    xts[b][q] = t_

            # --- S stage (stage-major: both batches before shifts) ---
            s_all = []
            for b in range(BPC):
                s_sb = wpool.tile([G, T], bf16, name=f"s{b}", tag="s")
                for q in range(2):
                    for cc in range(2):
                        ps = ppool.tile([G, 512], f32, name=f"ps{b}{q}{cc}",
                                        tag="ps")
                        for h in range(2):
                            nc.tensor.matmul(
                                ps[:],
                                cs_sb[:, G * (2 * b + h):G * (2 * b + h + 1)],
                                xts[b][q][:, 1024 * h + 512 * cc:
                                          1024 * h + 512 * cc + 512],
                                start=(h == 0), stop=(h == 1))
                        c0 = 1024 * q + 512 * cc
                        dst = s_sb[:, c0:c0 + 512]
                        if (q + cc) % 2 == 0:
                            nc.vector.tensor_copy(dst, ps[:])
                        else:
                            nc.scalar.activation(dst, ps[:], act_copy)
                s_all.append(s_sb)

            # --- shift replication: 7 per-k SBUF->SBUF DMAs per batch,
            # 4 on Sync + 3 on GpSimd (parallel queues) ---
            swin_all = []
            for b in range(BPC):
                swin = wpool.tile([K * G, T_OUT], bf16, name=f"swin{b}",
                                  tag="swin")
                for k in range(K):
                    eng = nc.sync if k % 2 == 0 else nc.gpsimd
                    eng.dma_start(swin[G * k:G * (k + 1), :],
                                  s_all[b][:, k:k + T_OUT])
                swin_all.append(swin)

            # --- conv + bias + store ---
            for b in range(BPC):
                osb = wpool.tile([D, T_OUT], bf16, name=f"osb{b}", tag="osb")
                for ci, (c0, c1) in enumerate(CONV_CHUNKS):
                    L = c1 - c0
                    po = opool.tile([D, 512], f32, name=f"po{b}{ci}", tag="po")
                    nc.tensor.matmul(po[:, :L], cs_sb[0:K * G, 64:128],
                                     swin_all[b][:, c0:c1],
                                     start=True, stop=True)
                    if ci % 2 == 0:
                        nc.vector.tensor_scalar(out=osb[:, c0:c1],
                                                in0=po[:, :L],
                                                scalar1=bc_sb[:, b:b + 1],
                                                scalar2=None, op0=add)
                    else:
                        nc.scalar.activation(osb[:, c0:c1], po[:, :L], act_id,
                                             bias=bc_sb[:, b:b + 1])
                nc.gpsimd.dma_start(out_ext[b], osb[:])

    nc.compile()
    return nc


def _get_nc(mm_dtype: str):
    if mm_dtype not in _COMPILED:
        _COMPILED[mm_dtype] = _build(mm_dtype)
    return _COMPILED[mm_dtype]


def _run(x, group_idxs, W, bias, mm_dtype=None, trace=False, tmpdir=None):
    from concourse.bass_utils import run_bass_kernel_spmd

    x = np.asarray(x, dtype=np.float32)
    gi = np.asarray(group_idxs)
    W = np.asarray(W, dtype=np.float32)
    bias = np.asarray(bias, dtype=np.float32)

    # x per core: [2, 256, 2048] -> [b, h, p, q, t'] -> [b, q, p, h, t']
    xr = x.reshape(BS // BPC, BPC, 2, 128, 2, 1024).transpose(0, 1, 4, 3, 2, 5)
    xr = np.ascontiguousarray(xr.reshape(BS // BPC, BPC, 2, 128, 2048)
                              ).astype(BF16)
    # one-hot M: [bs, ch] -> per core [128, 4*G] with col (2b+h)*G+g
    oh = (gi[..., None] == np.arange(G)).astype(np.float32)  # [bs, 256, 16]
    ohm = oh.reshape(BS // BPC, BPC, 2, 128, G).transpose(0, 3, 1, 2, 4)
    ohm = ohm.reshape(BS // BPC, 128, 4 * G)
    # ws[k*16+g, d] = W[g, d, k], zero-padded to 128 rows
    ws = np.zeros((128, D), dtype=np.float32)
    ws[:K * G] = W.transpose(2, 0, 1).reshape(K * G, D)
    cs = np.concatenate([ohm, np.broadcast_to(ws, (BS // BPC, 128, D))],
                        axis=2).astype(BF16)  # [cores, 128, 128]
    # bc[d, b] = count0[b] * bias[d]
    count0 = (gi == 0).sum(axis=1).astype(np.float32)  # [bs]
    bc = (count0[None, :] * bias[:, None]).astype(np.float32)  # [64, bs]
    bc = bc.reshape(D, BS // BPC, BPC).transpose(1, 0, 2)  # [cores, 64, 2]

    nc = _get_nc(mm_dtype or MM_DTYPE)
    in_maps = []
    for i in range(N_CORES):
        in_maps.append({
            "x": xr[i],
            "cs": np.ascontiguousarray(cs[i]),
            "bc": np.ascontiguousarray(bc[i]),
        })
    res = run_bass_kernel_spmd(nc, in_maps, core_ids=list(range(N_CORES)),
                               trace=trace, tmpdir=tmpdir)
    out = np.concatenate([np.asarray(r["out"], dtype=np.float32)
                          for r in res.results], axis=0)
    assert out.shape == (BS, D, T_OUT)
    return out, res


def kernel(x, group_idxs, W, bias):
    out, _ = _run(x, group_idxs, W, bias)
    return out
